# revision 3
# baseline (speedup 1.0000x reference)
"""CAM (channel attention) module kernel for Trainium2, 8 NeuronCores.

Reference computation (per sample, x: [C, N] with C=512, N=64*64):
    energy    = x @ x.T                      # [C, C] symmetric Gram matrix
    energy_n  = rowmax(energy) - energy
    att       = softmax(energy_n, axis=-1)
    out       = gamma * (att @ x) + x

Softmax shift-invariance: softmax(rowmax - e) == softmax(-e), stabilized
with the row-min m_i:  att[i,j] = exp(m_i - e_ij) / S_i,  S_i = sum_j.

Sharding: pure data parallel over batch B=16 -> 2 samples per core.

Precision: fp16 matmul operands (10 mantissa bits, same class as tf32;
measured ~1e-2 relative vs float64 at gamma=1), fp32 PSUM accumulation,
exact fp32 "+ x" epilogue so gamma=0 reproduces x bit-exactly. fp16 runs
the PE at 1 cycle/row for any moving width (fp32 transposes pay 2x, and
fp32r matmuls pay 4x below 256 wide) and enables fast weight load.

Per-core pipeline (2 samples):
  1. load xf natural fp32 in 8 interleaved column pieces; GpSimd casts
     each landed piece to fp16 (nat16); warmup matmuls keep the PE clock
     un-throttled (HAM) while the first pieces land
  2. per 128-col chunk k: PE-transpose the 4 channel blocks (fp16)
     -> PSUM, evacuate to xt16 (alternating ACT/DVE), then advance ALL
     four triangular Gram panels by one k step (wavefront) so the PE
     never queues idle work behind DMA-paced transposes
  3. energy is symmetric: row panel ci computes columns [128*ci : 512]
     (exact upper triangle); lower blocks are mirrored from finished
     panels via fp32 PE transposes of stashed SBUF copies
  4. m = rowmin(energy) (DVE); P16 = exp(m - e) with fused row-sum S
     (ACT, reads PSUM directly)
  5. PT = P.T @ diag(gamma/S) on the PE (folds softmax normalization AND
     gamma into the transpose), bi-outer so early panels' PT matmuls
     overlap the last panel's softmax
  6. mm2: out_tile = PT[bj][:,ci*128:].T @ nat16[bj] accumulated over bj
     (moving operand slices nat16 directly - no re-round copies), then
     epilogue out = psum + x in one DVE scalar_tensor_tensor pass
     (x stays exact fp32, so gamma=0 reproduces x bit-exactly)
  7. the first 512 columns of the next sample are preloaded + cast
     during mm2 so the boundary transposes never stall the PE
"""

import numpy as np

import concourse.bacc as bacc
import concourse.tile as tile
from concourse import mybir
from concourse.bass_utils import run_bass_kernel_spmd
from concourse.masks import make_identity

B, C, H, W = 16, 512, 64, 64
N = H * W
NCORES = 8
BPC = B // NCORES  # samples per core
CB = C // 128      # channel blocks (4)
NK = N // 128      # 128-wide n-chunks (32)
NT = N // 512      # 512-wide n-tiles (8)

F32 = mybir.dt.float32
F16 = mybir.dt.float16


def _emit(nc, tc, ctx, x, gamma, out):
    consts = ctx.enter_context(tc.tile_pool(name="consts", bufs=1))
    nat_pool = ctx.enter_context(tc.tile_pool(name="nat", bufs=CB + 1))
    nat16_pool = ctx.enter_context(tc.tile_pool(name="nat16", bufs=CB + 1))
    xfT_pool = ctx.enter_context(tc.tile_pool(name="xfT", bufs=NK))
    p_pool = ctx.enter_context(tc.tile_pool(name="p", bufs=CB))
    pt_pool = ctx.enter_context(tc.tile_pool(name="pt", bufs=CB))
    eblk_pool = ctx.enter_context(tc.tile_pool(name="eblk", bufs=6))
    d_pool = ctx.enter_context(tc.tile_pool(name="d", bufs=CB))
    small = ctx.enter_context(tc.tile_pool(name="small", bufs=4 * CB))
    outs_pool = ctx.enter_context(tc.tile_pool(name="outs", bufs=4))
    head_pool = ctx.enter_context(tc.tile_pool(name="head", bufs=CB))
    head16_pool = ctx.enter_context(tc.tile_pool(name="head16", bufs=CB))
    psum_e = ctx.enter_context(tc.tile_pool(name="psum_e", bufs=CB, space="PSUM"))
    psum_g = ctx.enter_context(tc.tile_pool(name="psum_g", bufs=4, space="PSUM"))

    identity = consts.tile([128, 128], F32)
    make_identity(nc, identity[:])
    id16 = consts.tile([128, 128], F16)
    nc.vector.tensor_copy(out=id16[:], in_=identity[:])
    g_sb = consts.tile([128, 1], F32)
    nc.gpsimd.dma_start(out=g_sb[:], in_=gamma[:].to_broadcast((128, 1)))

    head_tiles = {}
    for s in range(BPC):
        # ---- load natural layout; split + interleave so the first
        # transposes only wait on the first piece of each chunk; GpSimd
        # casts each landed piece to fp16 ----
        nat = [
            nat_pool.tile([128, N], F32, tag="nat", name=f"nat{s}_{c}")
            for c in range(CB)
        ]
        nat16 = [
            nat16_pool.tile([128, N], F16, tag="nat16", name=f"nat16_{s}_{c}")
            for c in range(CB)
        ]
        QN = N // 8
        for q in range(8):
            for c in range(CB):
                nc.sync.dma_start(
                    out=nat[c][:, QN * q : QN * (q + 1)],
                    in_=x[s, 128 * c : 128 * (c + 1), QN * q : QN * (q + 1)],
                )
                nc.gpsimd.tensor_copy(
                    out=nat16[c][:, QN * q : QN * (q + 1)],
                    in_=nat[c][:, QN * q : QN * (q + 1)],
                )

        if s == 0:
            # keep the PE busy (HAM warm) while the first load pieces land
            warm_ps = psum_g.tile([128, 128], F32, tag="g", name=f"warm{s}")
            for w in range(16):
                nc.tensor.matmul(warm_ps[:], id16[:], id16[:], start=(w == 0), stop=False)
            nc.tensor.matmul(warm_ps[:], id16[:], id16[:], start=False, stop=True)

        # ---- transpose + triangular Gram wavefront: per k-chunk, PE
        # transposes the 4 blocks, the chunk is evacuated to fp16, and
        # then ALL four row panels advance one accumulation step ----
        # panel ci covers columns [128*ci : 512] (upper triangle incl diag)
        head = head_tiles.get(s)
        xts = []
        e_ps = [
            psum_e.tile([128, C], F32, tag="e", name=f"e_ps{s}_{ci}")
            for ci in range(CB)
        ]
        for k in range(NK):
            t_ps = psum_g.tile([128, C], F16, tag="g")
            for c in range(CB):
                src_ap = (
                    head[c][:, 128 * k : 128 * (k + 1)]
                    if head is not None and k < 4
                    else nat16[c][:, 128 * k : 128 * (k + 1)]
                )
                nc.tensor.transpose(
                    t_ps[:, 128 * c : 128 * (c + 1)],
                    src_ap,
                    id16[:],
                )
            xt = xfT_pool.tile([128, C], F16, tag="xfT")
            if k % 2 == 0:
                nc.scalar.activation(
                    out=xt[:], in_=t_ps[:],
                    func=mybir.ActivationFunctionType.Copy,
                    bias=0.0, scale=1.0,
                )
            else:
                nc.vector.tensor_copy(out=xt[:], in_=t_ps[:])
            xts.append(xt)
            for ci in range(CB):
                lo = 128 * ci
                nc.tensor.matmul(
                    e_ps[ci][:, lo:C],
                    xt[:, lo : lo + 128],
                    xt[:, lo:C],
                    start=(k == 0),
                    stop=(k == NK - 1),
                )

        # ---- per row panel: mirror, softmax pieces ----
        e_blk = {}  # (ci, cj) -> SBUF f32 copy of energy block for mirroring
        p_t = []
        d_t = []
        for ci in range(CB):
            e = e_ps[ci]
            # stash SBUF copies of the blocks later row-panels will mirror
            for cj in range(ci + 1, CB):
                blk = eblk_pool.tile(
                    [128, 128], F32, tag="eblk", name=f"eblk{s}_{ci}_{cj}"
                )
                nc.vector.tensor_copy(
                    out=blk[:], in_=e[:, 128 * cj : 128 * (cj + 1)]
                )
                e_blk[(ci, cj)] = blk
            # mirror missing lower blocks from earlier panels
            for cj in range(ci):
                nc.tensor.transpose(
                    e[:, 128 * cj : 128 * (cj + 1)],
                    e_blk[(cj, ci)][:],
                    identity[:],
                )
            # softmax pieces: P = exp(m - e), S = rowsum, D = diag(gamma/S)
            m = small.tile([128, 1], F32, tag="m")
            nc.vector.tensor_reduce(
                out=m[:], in_=e[:], axis=mybir.AxisListType.X,
                op=mybir.AluOpType.min,
            )
            p = p_pool.tile([128, C], F16, tag="p")
            ssum = small.tile([128, 1], F32, tag="s")
            nc.scalar.activation(
                out=p[:], in_=e[:],
                func=mybir.ActivationFunctionType.Exp,
                bias=m[:], scale=-1.0, accum_out=ssum[:],
            )
            r = small.tile([128, 1], F32, tag="r")
            nc.vector.reciprocal(out=r[:], in_=ssum[:])
            gv = small.tile([128, 1], F32, tag="gv")
            nc.vector.tensor_mul(out=gv[:], in0=r[:], in1=g_sb[:])
            d = d_pool.tile([128, 128], F16, tag="d")
            nc.vector.tensor_scalar_mul(out=d[:], in0=identity[:], scalar1=gv[:])
            p_t.append(p)
            d_t.append(d)

        # ---- PT = P.T @ diag(gamma/S): PT[j, i] = gamma * att[i, j] ----
        # bi-outer: the PT matmuls for early row panels run while the last
        # panel's softmax is still on DVE/ACT
        ptps = [
            psum_g.tile([128, C], F32, tag="g", name=f"ptp{s}_{bj}")
            for bj in range(CB)
        ]
        for bi in range(CB):
            for bj in range(CB):
                nc.tensor.matmul(
                    ptps[bj][:, 128 * bi : 128 * (bi + 1)],
                    p_t[bi][:, 128 * bj : 128 * (bj + 1)],
                    d_t[bi][:],
                    start=True,
                    stop=True,
                )
        pt = []
        for bj in range(CB):
            ptt = pt_pool.tile([128, C], F16, tag="pt", name=f"ptt{s}_{bj}")
            nc.scalar.activation(
                out=ptt[:], in_=ptps[bj][:],
                func=mybir.ActivationFunctionType.Copy,
                bias=0.0, scale=1.0,
            )
            pt.append(ptt)

        # pre-load + cast the first 512 columns of the next sample so the
        # boundary transposes have fp16 data ready and the PE avoids an
        # idle window long enough to re-throttle the clock
        if s + 1 < BPC:
            head_tiles[s + 1] = []
            for c in range(CB):
                ht = head_pool.tile(
                    [128, 512], F32, tag="head", name=f"head{s + 1}_{c}"
                )
                nc.sync.dma_start(
                    out=ht[:], in_=x[s + 1, 128 * c : 128 * (c + 1), 0:512]
                )
                ht16 = head16_pool.tile(
                    [128, 512], F16, tag="head16", name=f"head16_{s + 1}_{c}"
                )
                nc.gpsimd.tensor_copy(out=ht16[:], in_=ht[:])
                head_tiles[s + 1].append(ht16)

        # ---- out = PT.T @ xf16 + x ----
        for nt in range(NT):
            for ci in range(CB):
                ops = psum_g.tile([128, 512], F32, tag="g")
                for bj in range(CB):
                    nc.tensor.matmul(
                        ops[:],
                        pt[bj][:, 128 * ci : 128 * (ci + 1)],
                        nat16[bj][:, 512 * nt : 512 * (nt + 1)],
                        start=(bj == 0),
                        stop=(bj == CB - 1),
                    )
                o_sb = outs_pool.tile([128, 512], F32, tag="o")
                nc.vector.scalar_tensor_tensor(
                    out=o_sb[:],
                    in0=ops[:],
                    scalar=1.0,
                    in1=nat[ci][:, 512 * nt : 512 * (nt + 1)],
                    op0=mybir.AluOpType.bypass,
                    op1=mybir.AluOpType.add,
                )
                nc.sync.dma_start(
                    out=out[
                        s, 128 * ci : 128 * (ci + 1), 512 * nt : 512 * (nt + 1)
                    ],
                    in_=o_sb[:],
                )


_NC_CACHE = None


def _build():
    global _NC_CACHE
    if _NC_CACHE is not None:
        return _NC_CACHE
    from contextlib import ExitStack

    nc = bacc.Bacc("TRN2", target_bir_lowering=False)
    x = nc.dram_tensor("x", [BPC, C, N], F32, kind="ExternalInput")
    gamma = nc.dram_tensor("gamma", [1, 1], F32, kind="ExternalInput")
    out = nc.dram_tensor("out", [BPC, C, N], F32, kind="ExternalOutput")
    with tile.TileContext(nc) as tc:
        with ExitStack() as ctx:
            _emit(nc, tc, ctx, x[:], gamma[:], out[:])
    nc.compile()
    _NC_CACHE = nc
    return nc


def kernel(x, gamma):
    x = np.ascontiguousarray(np.asarray(x, dtype=np.float32))
    gamma = np.ascontiguousarray(np.asarray(gamma, dtype=np.float32))
    assert x.shape == (B, C, H, W), x.shape
    xf = x.reshape(B, C, N)
    nc = _build()
    in_maps = [
        {
            "x": xf[c * BPC : (c + 1) * BPC],
            "gamma": gamma.reshape(1, 1),
        }
        for c in range(NCORES)
    ]
    res = run_bass_kernel_spmd(nc, in_maps, core_ids=list(range(NCORES)))
    out = np.concatenate([res.results[c]["out"] for c in range(NCORES)], axis=0)
    return out.reshape(B, C, H, W)


# revision 5
# speedup vs baseline: 1.3033x; 1.3033x over previous
"""CAM (channel attention) module kernel for Trainium2, 8 NeuronCores.

Reference computation (per sample, x: [C, N] with C=512, N=64*64):
    energy    = x @ x.T                      # [C, C] symmetric Gram matrix
    energy_n  = rowmax(energy) - energy
    att       = softmax(energy_n, axis=-1)
    out       = gamma * (att @ x) + x

Softmax shift-invariance: softmax(rowmax - e) == softmax(-e), stabilized
with the row-min m_i:  att[i,j] = exp(m_i - e_ij) / S_i,  S_i = sum_j.

Sharding: pure data parallel over batch B=16 -> 2 samples per core.

Precision: the Gram matrix is computed from fp16 operands (10 mantissa
bits; ~1e-2 relative vs float64 at gamma=1), mm2 runs in fp32r; all
accumulation is fp32 PSUM and the "+ x" epilogue is exact fp32, so
gamma=0 reproduces x bit-exactly. fp16 runs the PE at 1 cycle/row for
ANY moving width (fp32r pays 4x below 256 wide), which makes the exact
128-block triangle + 128-wide PT matmuls cheap. The natural-layout
tiles are consumed through f32r *bitcasts* (f32r is bit-identical to
f32), so no conversion pass is needed anywhere: f32 -> fp16 rounding
happens for free inside the transpose-PSUM evacuations.

Per-core pipeline (2 samples):
  1. load xf natural fp32 in 8 interleaved column pieces; warmup
     matmuls keep the PE clock un-throttled (HAM) while they land
  2. per 128-col chunk k: PE-transpose the 4 channel blocks (f32r view,
     1.5 cyc/row) -> PSUM, evacuate to fp16 xt (alternating ACT/DVE),
     then advance ALL four triangular Gram panels by one k step
     (wavefront) so the PE never queues idle work behind DMA pacing
  3. energy is symmetric: row panel ci computes columns [128*ci : 512]
     (exact upper triangle); lower blocks are mirrored from finished
     panels via PE transposes of stashed SBUF copies
  4. m = rowmin(energy) (DVE); P16 = exp(m - e) with fused row-sum S
     (ACT, reads PSUM directly)
  5. PT = P.T @ diag(gamma/S) on the PE in fp16 (folds softmax
     normalization AND gamma into the transpose), bi-outer so early
     panels' PT matmuls overlap the last panel's softmax; evacuated as
     f32r
  6. mm2: out_tile = PT[bj][:,ci*128:].T @ natR[bj] accumulated over bj
     (moving operand is the f32r view of nat - no copies), epilogue
     out = psum + x in one DVE scalar_tensor_tensor pass
  7. the first 512 columns of the next sample are preloaded during mm2
     so the boundary transposes never stall the PE
"""

import numpy as np

import concourse.bacc as bacc
import concourse.tile as tile
from concourse import mybir
from concourse.bass_utils import run_bass_kernel_spmd
from concourse.masks import make_identity

B, C, H, W = 16, 512, 64, 64
N = H * W
NCORES = 8
BPC = B // NCORES  # samples per core
CB = C // 128      # channel blocks (4)
NK = N // 128      # 128-wide n-chunks (32)
NT = N // 512      # 512-wide n-tiles (8)

F32 = mybir.dt.float32
F32R = mybir.dt.float32r
F16 = mybir.dt.float16


def _emit(nc, tc, ctx, x, gamma, out):
    consts = ctx.enter_context(tc.tile_pool(name="consts", bufs=1))
    nat_pool = ctx.enter_context(tc.tile_pool(name="nat", bufs=CB + 1))
    xfT_pool = ctx.enter_context(tc.tile_pool(name="xfT", bufs=NK))
    p_pool = ctx.enter_context(tc.tile_pool(name="p", bufs=CB))
    pt_pool = ctx.enter_context(tc.tile_pool(name="pt", bufs=CB))
    eblk_pool = ctx.enter_context(tc.tile_pool(name="eblk", bufs=6))
    d_pool = ctx.enter_context(tc.tile_pool(name="d", bufs=CB))
    small = ctx.enter_context(tc.tile_pool(name="small", bufs=4 * CB))
    outs_pool = ctx.enter_context(tc.tile_pool(name="outs", bufs=4))
    head_pool = ctx.enter_context(tc.tile_pool(name="head", bufs=CB))
    psum_e = ctx.enter_context(tc.tile_pool(name="psum_e", bufs=CB, space="PSUM"))
    psum_g = ctx.enter_context(tc.tile_pool(name="psum_g", bufs=4, space="PSUM"))

    identity = consts.tile([128, 128], F32)
    make_identity(nc, identity[:])
    idR_t = consts.tile([128, 128], F32R)
    nc.vector.tensor_copy(out=idR_t[:], in_=identity[:])
    idR = idR_t[:]
    id16 = consts.tile([128, 128], F16)
    nc.vector.tensor_copy(out=id16[:], in_=identity[:])
    g_sb = consts.tile([128, 1], F32)
    nc.gpsimd.dma_start(out=g_sb[:], in_=gamma[:].to_broadcast((128, 1)))

    head_tiles = {}
    for s in range(BPC):
        # ---- load natural layout; split + interleave so the first
        # transposes only wait on the first piece of each chunk ----
        nat = [
            nat_pool.tile([128, N], F32R, tag="nat", name=f"nat{s}_{c}")
            for c in range(CB)
        ]
        QN = N // 8
        for q in range(8):
            for c in range(CB):
                nc.sync.dma_start(
                    out=nat[c][:, QN * q : QN * (q + 1)],
                    in_=x[s, 128 * c : 128 * (c + 1), QN * q : QN * (q + 1)],
                )

        if s == 0:
            # keep the PE busy (HAM warm) while the first load pieces land
            warm_ps = psum_g.tile([128, 128], F32, tag="g", name=f"warm{s}")
            for w in range(16):
                nc.tensor.matmul(warm_ps[:], id16[:], id16[:], start=(w == 0), stop=False)
            nc.tensor.matmul(warm_ps[:], id16[:], id16[:], start=False, stop=True)

        # ---- transpose + triangular Gram wavefront: per k-chunk, PE
        # transposes the 4 blocks (f32r), the chunk is evacuated to fp16,
        # then ALL four row panels advance one accumulation step ----
        # panel ci covers columns [128*ci : 512] (upper triangle incl diag)
        head = head_tiles.get(s)
        xts = []
        e_ps = [
            psum_e.tile([128, C], F32, tag="e", name=f"e_ps{s}_{ci}")
            for ci in range(CB)
        ]
        for k in range(NK):
            t_ps = psum_g.tile([128, C], F32R, tag="g")
            for c in range(CB):
                src_ap = (
                    head[c][:, 128 * k : 128 * (k + 1)]
                    if head is not None and k < 4
                    else nat[c][:, 128 * k : 128 * (k + 1)]
                )
                nc.tensor.transpose(
                    t_ps[:, 128 * c : 128 * (c + 1)],
                    src_ap,
                    idR,
                )
            xt = xfT_pool.tile([128, C], F16, tag="xfT")
            if k % 2 == 0:
                nc.scalar.activation(
                    out=xt[:], in_=t_ps[:].bitcast(F32),
                    func=mybir.ActivationFunctionType.Copy,
                    bias=0.0, scale=1.0,
                )
            else:
                nc.vector.tensor_copy(out=xt[:], in_=t_ps[:].bitcast(F32))
            xts.append(xt)
            for ci in range(CB):
                lo = 128 * ci
                nc.tensor.matmul(
                    e_ps[ci][:, lo:C],
                    xt[:, lo : lo + 128],
                    xt[:, lo:C],
                    start=(k == 0),
                    stop=(k == NK - 1),
                )

        # ---- per row panel: mirror, softmax pieces ----
        e_blk = {}  # (ci, cj) -> SBUF f32 copy of energy block for mirroring
        p_t = []
        d_t = []
        for ci in range(CB):
            e = e_ps[ci]
            # stash SBUF copies of the blocks later row-panels will mirror
            for cj in range(ci + 1, CB):
                blk = eblk_pool.tile(
                    [128, 128], F32, tag="eblk", name=f"eblk{s}_{ci}_{cj}"
                )
                nc.vector.tensor_copy(
                    out=blk[:], in_=e[:, 128 * cj : 128 * (cj + 1)]
                )
                e_blk[(ci, cj)] = blk
            # mirror missing lower blocks from earlier panels
            for cj in range(ci):
                nc.tensor.transpose(
                    e[:, 128 * cj : 128 * (cj + 1)],
                    e_blk[(cj, ci)][:],
                    identity[:],
                )
            # softmax pieces: P = exp(m - e), S = rowsum, D = diag(gamma/S)
            m = small.tile([128, 1], F32, tag="m")
            nc.vector.tensor_reduce(
                out=m[:], in_=e[:], axis=mybir.AxisListType.X,
                op=mybir.AluOpType.min,
            )
            p = p_pool.tile([128, C], F16, tag="p")
            ssum = small.tile([128, 1], F32, tag="s")
            nc.scalar.activation(
                out=p[:], in_=e[:],
                func=mybir.ActivationFunctionType.Exp,
                bias=m[:], scale=-1.0, accum_out=ssum[:],
            )
            r = small.tile([128, 1], F32, tag="r")
            nc.vector.reciprocal(out=r[:], in_=ssum[:])
            gv = small.tile([128, 1], F32, tag="gv")
            nc.vector.tensor_mul(out=gv[:], in0=r[:], in1=g_sb[:])
            d = d_pool.tile([128, 128], F16, tag="d")
            nc.vector.tensor_scalar_mul(out=d[:], in0=identity[:], scalar1=gv[:])
            p_t.append(p)
            d_t.append(d)

        # ---- PT = P.T @ diag(gamma/S): PT[j, i] = gamma * att[i, j] ----
        # bi-outer: the PT matmuls for early row panels run while the last
        # panel's softmax is still on DVE/ACT
        ptps = [
            psum_g.tile([128, C], F32, tag="g", name=f"ptp{s}_{bj}")
            for bj in range(CB)
        ]
        for bi in range(CB):
            for bj in range(CB):
                nc.tensor.matmul(
                    ptps[bj][:, 128 * bi : 128 * (bi + 1)],
                    p_t[bi][:, 128 * bj : 128 * (bj + 1)],
                    d_t[bi][:],
                    start=True,
                    stop=True,
                )
        pt = []
        for bj in range(CB):
            ptt = pt_pool.tile([128, C], F32R, tag="pt", name=f"ptt{s}_{bj}")
            nc.scalar.activation(
                out=ptt[:], in_=ptps[bj][:],
                func=mybir.ActivationFunctionType.Copy,
                bias=0.0, scale=1.0,
            )
            pt.append(ptt)

        # pre-load the first 512 columns of the next sample so the
        # boundary transposes have data ready and the PE avoids an idle
        # window long enough to re-throttle the clock
        if s + 1 < BPC:
            head_tiles[s + 1] = []
            for c in range(CB):
                ht = head_pool.tile(
                    [128, 512], F32R, tag="head", name=f"head{s + 1}_{c}"
                )
                nc.sync.dma_start(
                    out=ht[:], in_=x[s + 1, 128 * c : 128 * (c + 1), 0:512]
                )
                head_tiles[s + 1].append(ht[:])

        # ---- out = PT.T @ xf + x ----
        for nt in range(NT):
            for ci in range(CB):
                ops = psum_g.tile([128, 512], F32, tag="g")
                for bj in range(CB):
                    nc.tensor.matmul(
                        ops[:],
                        pt[bj][:, 128 * ci : 128 * (ci + 1)],
                        nat[bj][:, 512 * nt : 512 * (nt + 1)],
                        start=(bj == 0),
                        stop=(bj == CB - 1),
                    )
                o_sb = outs_pool.tile([128, 512], F32, tag="o")
                nc.vector.scalar_tensor_tensor(
                    out=o_sb[:],
                    in0=ops[:],
                    scalar=1.0,
                    in1=nat[ci][:, 512 * nt : 512 * (nt + 1)],
                    op0=mybir.AluOpType.bypass,
                    op1=mybir.AluOpType.add,
                )
                nc.sync.dma_start(
                    out=out[
                        s, 128 * ci : 128 * (ci + 1), 512 * nt : 512 * (nt + 1)
                    ],
                    in_=o_sb[:],
                )


_NC_CACHE = None


def _build():
    global _NC_CACHE
    if _NC_CACHE is not None:
        return _NC_CACHE
    from contextlib import ExitStack

    nc = bacc.Bacc("TRN2", target_bir_lowering=False)
    x = nc.dram_tensor("x", [BPC, C, N], F32R, kind="ExternalInput")
    gamma = nc.dram_tensor("gamma", [1, 1], F32, kind="ExternalInput")
    out = nc.dram_tensor("out", [BPC, C, N], F32, kind="ExternalOutput")
    with tile.TileContext(nc) as tc:
        with ExitStack() as ctx:
            _emit(nc, tc, ctx, x[:], gamma[:], out[:])
    nc.compile()
    _NC_CACHE = nc
    return nc


def kernel(x, gamma):
    x = np.ascontiguousarray(np.asarray(x, dtype=np.float32))
    gamma = np.ascontiguousarray(np.asarray(gamma, dtype=np.float32))
    assert x.shape == (B, C, H, W), x.shape
    xf = x.reshape(B, C, N)
    nc = _build()
    in_maps = [
        {
            "x": xf[c * BPC : (c + 1) * BPC],
            "gamma": gamma.reshape(1, 1),
        }
        for c in range(NCORES)
    ]
    res = run_bass_kernel_spmd(nc, in_maps, core_ids=list(range(NCORES)))
    out = np.concatenate([res.results[c]["out"] for c in range(NCORES)], axis=0)
    return out.reshape(B, C, H, W)


# revision 6
# speedup vs baseline: 1.3770x; 1.0566x over previous
"""CAM (channel attention) module kernel for Trainium2, 8 NeuronCores.

Reference computation (per sample, x: [C, N] with C=512, N=64*64):
    energy    = x @ x.T                      # [C, C] symmetric Gram matrix
    energy_n  = rowmax(energy) - energy
    att       = softmax(energy_n, axis=-1)
    out       = gamma * (att @ x) + x

Softmax shift-invariance: softmax(rowmax - e) == softmax(-e), stabilized
with the row-min m_i:  att[i,j] = exp(m_i - e_ij) / S_i,  S_i = sum_j.

Sharding: pure data parallel over batch B=16 -> 2 samples per core.

Precision: the Gram matrix is computed from fp16 operands (10 mantissa
bits; ~1e-2 relative vs float64 at gamma=1), mm2 runs in fp32r; all
accumulation is fp32 PSUM and the "+ x" epilogue is exact fp32, so
gamma=0 reproduces x bit-exactly. fp16 runs the PE at 1 cycle/row for
ANY moving width (fp32r pays 4x below 256 wide), which makes the exact
128-block triangle + 128-wide PT matmuls cheap. The natural-layout
tiles are DECLARED f32r (bit-identical to f32), so no rounding pass is
needed anywhere: f32 -> fp16 rounding happens for free inside the
transpose-PSUM evacuations, and the PE truncates f32r mantissas itself.

Per-core pipeline (2 samples):
  1. load xf natural in 8 interleaved column pieces on the sync queue
     (output stores go on the otherwise-idle GpSimd queue so the next
     sample's loads are never stuck behind them); warmup matmuls keep
     the PE clock un-throttled (HAM) while the first pieces land
  2. per 128-col chunk k: PE-transpose the 4 channel blocks (f32r,
     1.5 cyc/row) -> PSUM, evacuate to fp16 xt (alternating ACT/DVE),
     then advance ALL four triangular Gram panels by one k step
     (wavefront) so the PE never queues idle work behind DMA pacing
  3. energy is symmetric: row panel ci computes columns [128*ci : 512]
     (exact upper triangle); lower blocks are mirrored from finished
     panels via PE transposes of stashed SBUF copies
  4. software-pipelined tail: for each ci - softmax(ci) [rowmin on DVE,
     P16 = exp(m - e) with fused row-sum on ACT, D = diag(gamma/S) in
     one fused DVE tensor_scalar], then mm2 block ci-1, then
     PT(ci) = P[ci].T @ D[ci] (4 fp16 128-wide matmuls, evacuated f32r
     in bj-grouped layout). mm2 for output block ci only needs
     softmax(ci), so each softmax hides under the previous mm2 block;
     only softmax(0)'s ~2.5us latency is exposed per sample
  5. mm2 block ci: out[ci] = sum_bj PT_ci[bj].T @ nat[bj] (moving
     operand is the f32r nat - no copies), epilogue out = psum + x in
     one DVE scalar_tensor_tensor pass (x read as exact fp32 bits)
  6. the next sample's FULL load is emitted before mm2 so it streams in
     during the ~14us of mm2 blocks and the boundary never stalls
"""

import numpy as np

import concourse.bacc as bacc
import concourse.tile as tile
from concourse import mybir
from concourse.bass_utils import run_bass_kernel_spmd
from concourse.masks import make_identity

B, C, H, W = 16, 512, 64, 64
N = H * W
NCORES = 8
BPC = B // NCORES  # samples per core
CB = C // 128      # channel blocks (4)
NK = N // 128      # 128-wide n-chunks (32)
NT = N // 512      # 512-wide n-tiles (8)

F32 = mybir.dt.float32
F32R = mybir.dt.float32r
F16 = mybir.dt.float16


def _emit(nc, tc, ctx, x, gamma, out):
    consts = ctx.enter_context(tc.tile_pool(name="consts", bufs=1))
    nat_pool = ctx.enter_context(tc.tile_pool(name="nat", bufs=2 * CB))
    xfT_pool = ctx.enter_context(tc.tile_pool(name="xfT", bufs=NK))
    p_pool = ctx.enter_context(tc.tile_pool(name="p", bufs=CB))
    pt_pool = ctx.enter_context(tc.tile_pool(name="pt", bufs=CB))
    eblk_pool = ctx.enter_context(tc.tile_pool(name="eblk", bufs=6))
    d_pool = ctx.enter_context(tc.tile_pool(name="d", bufs=CB))
    small = ctx.enter_context(tc.tile_pool(name="small", bufs=4 * CB))
    outs_pool = ctx.enter_context(tc.tile_pool(name="outs", bufs=4))
    psum_e = ctx.enter_context(tc.tile_pool(name="psum_e", bufs=CB, space="PSUM"))
    psum_g = ctx.enter_context(tc.tile_pool(name="psum_g", bufs=4, space="PSUM"))

    identity = consts.tile([128, 128], F32)
    make_identity(nc, identity[:])
    idR_t = consts.tile([128, 128], F32R)
    nc.vector.tensor_copy(out=idR_t[:], in_=identity[:])
    idR = idR_t[:]
    id16 = consts.tile([128, 128], F16)
    nc.vector.tensor_copy(out=id16[:], in_=identity[:])
    g_sb = consts.tile([128, 1], F32)
    nc.gpsimd.dma_start(out=g_sb[:], in_=gamma[:].to_broadcast((128, 1)))

    QN = N // 8

    def load_sample(s):
        nat = [
            nat_pool.tile([128, N], F32R, tag="nat", name=f"nat{s}_{c}")
            for c in range(CB)
        ]
        for q in range(8):
            for c in range(CB):
                nc.sync.dma_start(
                    out=nat[c][:, QN * q : QN * (q + 1)],
                    in_=x[s, 128 * c : 128 * (c + 1), QN * q : QN * (q + 1)],
                )
        return nat

    nats = {0: load_sample(0)}
    for s in range(BPC):
        nat = nats.pop(s)

        if s == 0:
            # keep the PE busy (HAM warm) while the first load pieces land
            warm_ps = psum_g.tile([128, 128], F32, tag="g", name=f"warm{s}")
            for w in range(16):
                nc.tensor.matmul(warm_ps[:], id16[:], id16[:], start=(w == 0), stop=False)
            nc.tensor.matmul(warm_ps[:], id16[:], id16[:], start=False, stop=True)

        # ---- transpose + triangular Gram wavefront: per k-chunk, PE
        # transposes the 4 blocks (f32r), the chunk is evacuated to fp16,
        # then ALL four row panels advance one accumulation step ----
        # panel ci covers columns [128*ci : 512] (upper triangle incl diag)
        e_ps = [
            psum_e.tile([128, C], F32, tag="e", name=f"e_ps{s}_{ci}")
            for ci in range(CB)
        ]
        for k in range(NK):
            t_ps = psum_g.tile([128, C], F32R, tag="g")
            for c in range(CB):
                nc.tensor.transpose(
                    t_ps[:, 128 * c : 128 * (c + 1)],
                    nat[c][:, 128 * k : 128 * (k + 1)],
                    idR,
                )
            xt = xfT_pool.tile([128, C], F16, tag="xfT")
            if k % 2 == 0:
                nc.scalar.activation(
                    out=xt[:], in_=t_ps[:].bitcast(F32),
                    func=mybir.ActivationFunctionType.Copy,
                    bias=0.0, scale=1.0,
                )
            else:
                nc.vector.tensor_copy(out=xt[:], in_=t_ps[:].bitcast(F32))
            for ci in range(CB):
                lo = 128 * ci
                nc.tensor.matmul(
                    e_ps[ci][:, lo:C],
                    xt[:, lo : lo + 128],
                    xt[:, lo:C],
                    start=(k == 0),
                    stop=(k == NK - 1),
                )

        # ---- software-pipelined softmax(ci) / mm2(ci-1) / PT(ci) ----
        e_blk = {}  # (ci, cj) -> SBUF f32 copy of energy block for mirroring
        pt_c = []   # per ci: PT columns 128*ci..128*(ci+1), bj-grouped

        def mm2_block(ci):
            # out[ci] = sum_bj PT_ci[bj].T @ nat[bj] ; epilogue += x
            ptc = pt_c[ci]
            for nt in range(NT):
                ops = psum_g.tile([128, 512], F32, tag="g")
                for bj in range(CB):
                    nc.tensor.matmul(
                        ops[:],
                        ptc[:, 128 * bj : 128 * (bj + 1)],
                        nat[bj][:, 512 * nt : 512 * (nt + 1)],
                        start=(bj == 0),
                        stop=(bj == CB - 1),
                    )
                o_sb = outs_pool.tile([128, 512], F32, tag="o")
                nc.vector.scalar_tensor_tensor(
                    out=o_sb[:],
                    in0=ops[:],
                    scalar=1.0,
                    in1=nat[ci][:, 512 * nt : 512 * (nt + 1)].bitcast(F32),
                    op0=mybir.AluOpType.bypass,
                    op1=mybir.AluOpType.add,
                )
                nc.gpsimd.dma_start(
                    out=out[
                        s, 128 * ci : 128 * (ci + 1), 512 * nt : 512 * (nt + 1)
                    ],
                    in_=o_sb[:],
                )

        for ci in range(CB):
            e = e_ps[ci]
            # stash SBUF copies of the blocks later row-panels will mirror
            for cj in range(ci + 1, CB):
                blk = eblk_pool.tile(
                    [128, 128], F32, tag="eblk", name=f"eblk{s}_{ci}_{cj}"
                )
                nc.vector.tensor_copy(
                    out=blk[:], in_=e[:, 128 * cj : 128 * (cj + 1)]
                )
                e_blk[(ci, cj)] = blk
            # mirror missing lower blocks from earlier panels
            for cj in range(ci):
                nc.tensor.transpose(
                    e[:, 128 * cj : 128 * (cj + 1)],
                    e_blk[(cj, ci)][:],
                    identity[:],
                )
            # softmax pieces: P = exp(m - e), S = rowsum, D = diag(gamma/S)
            m = small.tile([128, 1], F32, tag="m")
            nc.vector.tensor_reduce(
                out=m[:], in_=e[:], axis=mybir.AxisListType.X,
                op=mybir.AluOpType.min,
            )
            p = p_pool.tile([128, C], F16, tag="p")
            ssum = small.tile([128, 1], F32, tag="s")
            nc.scalar.activation(
                out=p[:], in_=e[:],
                func=mybir.ActivationFunctionType.Exp,
                bias=m[:], scale=-1.0, accum_out=ssum[:],
            )
            r = small.tile([128, 1], F32, tag="r")
            nc.vector.reciprocal(out=r[:], in_=ssum[:])
            d = d_pool.tile([128, 128], F16, tag="d")
            nc.vector.tensor_scalar(
                out=d[:], in0=identity[:], scalar1=r[:], scalar2=g_sb[:],
                op0=mybir.AluOpType.mult, op1=mybir.AluOpType.mult,
            )

            # overlap: previous output block's mm2 runs while this
            # panel's softmax is still on DVE/ACT
            if ci > 0:
                mm2_block(ci - 1)
            elif s + 1 < BPC:
                # next sample's full load streams in during the mm2 blocks
                nats[s + 1] = load_sample(s + 1)

            # PT(ci) = P[ci].T @ D[ci]: [j, i] = gamma * att[i, j] for
            # i in block ci, laid out bj-grouped along the free axis
            ptp = psum_g.tile([128, C], F32, tag="g", name=f"ptp{s}_{ci}")
            for bj in range(CB):
                nc.tensor.matmul(
                    ptp[:, 128 * bj : 128 * (bj + 1)],
                    p[:, 128 * bj : 128 * (bj + 1)],
                    d[:],
                    start=True,
                    stop=True,
                )
            ptc = pt_pool.tile([128, C], F32R, tag="pt", name=f"ptc{s}_{ci}")
            nc.scalar.activation(
                out=ptc[:], in_=ptp[:],
                func=mybir.ActivationFunctionType.Copy,
                bias=0.0, scale=1.0,
            )
            pt_c.append(ptc)

        mm2_block(CB - 1)


_NC_CACHE = None


def _build():
    global _NC_CACHE
    if _NC_CACHE is not None:
        return _NC_CACHE
    from contextlib import ExitStack

    nc = bacc.Bacc("TRN2", target_bir_lowering=False)
    x = nc.dram_tensor("x", [BPC, C, N], F32R, kind="ExternalInput")
    gamma = nc.dram_tensor("gamma", [1, 1], F32, kind="ExternalInput")
    out = nc.dram_tensor("out", [BPC, C, N], F32, kind="ExternalOutput")
    with tile.TileContext(nc) as tc:
        with ExitStack() as ctx:
            _emit(nc, tc, ctx, x[:], gamma[:], out[:])
    nc.compile()
    _NC_CACHE = nc
    return nc


def kernel(x, gamma):
    x = np.ascontiguousarray(np.asarray(x, dtype=np.float32))
    gamma = np.ascontiguousarray(np.asarray(gamma, dtype=np.float32))
    assert x.shape == (B, C, H, W), x.shape
    xf = x.reshape(B, C, N)
    nc = _build()
    in_maps = [
        {
            "x": xf[c * BPC : (c + 1) * BPC],
            "gamma": gamma.reshape(1, 1),
        }
        for c in range(NCORES)
    ]
    res = run_bass_kernel_spmd(nc, in_maps, core_ids=list(range(NCORES)))
    out = np.concatenate([res.results[c]["out"] for c in range(NCORES)], axis=0)
    return out.reshape(B, C, H, W)
